# revision 41
# baseline (speedup 1.0000x reference)
"""Compressed Interaction Network (CIN) forward on 8 Trainium2 NeuronCores.

Math (per batch item, m=32 fields, d=64 embed, H=256 hidden):
    x0 = x[i]                          # (m, d)
    h  = x0
    layer l in 0..2:
        z = outer(x0, h) over d        # (m*n, d), z[(a,b),:] = x0[a,:]*h[b,:]
        y = relu(W_l^T z + b_l)        # (H, d)
        xcur, h = split_half(y) (layers 0,1); xcur = h = y (layer 2)
    f = concat(xcur_0, xcur_1, xcur_2) # (512, d)
    out[i] = sum_d(f) @ fc_W + fc_b    # scalar

Mapping: batch 1024 -> 8 cores x 128 items, 16 groups of 8 items per core.

v2 design notes (vs the earlier per-group-serial version):
 - Layer 0 exploits z0 symmetry: z0[(a,b)] = z0[(b,a)], so W0 is folded on
   the host to 528 = 32*33/2 unique pair rows (padded to 640 = 5 k-chunks
   of 128).  The two z0 operands are host-gathered tensors (xqa, xqb) laid
   out so ONE full-128-partition DVE multiply builds a whole group's z0.
 - Layers 1/2: z chunks built on VectorE in fp16 (2x mode) from h x Bg
   (x broadcast to all partitions by one DMA per group); each chunk is
   consumed immediately by two interleaved matmuls (oc0/oc1) so at most a
   couple of chunks are alive.
 - Software pipeline over groups keeps the PE queue dense (HAM stays at
   K=8/8).  PE order per iteration i: [L1(i)][L0(i+1)][L2(i)].  The ACT of
   h2(i) + DVE build of z2(i) hide under L0(i+1); ACT h1(i+1) + build of
   z1(i+1) hide under L2(i).
 - Bias+ReLU fused in the PSUM->SBUF ACT; per-item d-sums for the final FC
   are 4 DVE X-axis reduces per group (fp32 out) instead of 512 per-item
   ScalarE accum ops.
 - Final dot: PE matmul of [128,1] fc weight chunks against [128,128] sums.
"""

import numpy as np

import concourse.bass as bass
import concourse.tile as tile
from concourse import mybir
from concourse.bass_utils import run_bass_kernel_spmd

N_CORES = 8
B_TOTAL = 1024
B_CORE = B_TOTAL // N_CORES  # 128
M = 32  # num fields
D = 64  # embed dim
H = 256  # conv output channels
GROUP = 8  # items per group (512 moving columns)
N_GROUPS = B_CORE // GROUP  # 16
MD = M * D  # 2048, elements per item row
NP0 = (M * (M + 1)) // 2  # 528 unique symmetric pairs in layer 0
C0 = (NP0 + 127) // 128  # 5 k-chunks of 128 (padded with zero weight rows)

F16 = mybir.dt.float16
F32 = mybir.dt.float32
RELU = mybir.ActivationFunctionType.Relu
IDENT = mybir.ActivationFunctionType.Identity
AX_X = mybir.AxisListType.X


def build():
    nc = bass.Bass()
    xh = nc.declare_dram_parameter("xh", [B_CORE, M, D], F16, isOutput=False)
    # layer-0 symmetric-pair operands, partition-major per group so one DMA is
    # 128 contiguous 5KB runs (descriptor-rate matters at startup):
    # xqa[g, p, (c, i, d)] = x_{8g+i}[amap[c*128+p], d]
    XQF = C0 * GROUP * D  # 2560 free elems per partition per group
    xqa = nc.declare_dram_parameter("xqa", [N_GROUPS, 128, XQF], F16, isOutput=False)
    xqb = nc.declare_dram_parameter("xqb", [N_GROUPS, 128, XQF], F16, isOutput=False)
    # conv weights host-pretransposed to [k-part, chunk, H] (contiguous load)
    w0s = nc.declare_dram_parameter("w0s", [128, C0, H], F16, isOutput=False)
    w1 = nc.declare_dram_parameter("w1", [128, 32, H], F16, isOutput=False)
    w2 = nc.declare_dram_parameter("w2", [128, 32, H], F16, isOutput=False)
    bia = nc.declare_dram_parameter("bia", [128, 3, 2], F32, isOutput=False)
    fcw = nc.declare_dram_parameter("fcw", [128, 4], F32, isOutput=False)
    fcb = nc.declare_dram_parameter("fcb", [1, 1], F32, isOutput=False)
    out = nc.declare_dram_parameter("out", [B_CORE, 1], F32, isOutput=True)

    with tile.TileContext(nc) as tc:
        with (
            tc.tile_pool(name="consts", bufs=1) as consts,
            tc.tile_pool(name="bgpool", bufs=2) as bgpool,
            tc.tile_pool(name="xqpool", bufs=2) as xqpool,
            tc.tile_pool(name="z0pool", bufs=2) as z0pool,
            tc.tile_pool(name="zpool", bufs=12) as zpool,
            tc.tile_pool(name="zspool", bufs=4) as zspool,
            tc.tile_pool(name="hpool", bufs=4) as hpool,
            tc.tile_pool(name="rypool", bufs=8) as rypool,
            tc.tile_pool(name="spool", bufs=1) as spool,
            tc.tile_pool(name="ppool", bufs=6, space="PSUM") as ppool,
            tc.tile_pool(name="fcp", bufs=1, space="PSUM") as fcp,
        ):
            # const tiles declared up front; DMAs are issued below spread
            # across engine queues in consumption order so the pipeline can
            # start ~40us earlier (startup DMA is descriptor/bandwidth bound).
            w0s_sb = consts.tile([128, C0, H], F16, tag="w0s")
            # w1 split in two tiles so the first 16 k-chunks' LDWEIGHTS only
            # wait on the first half's DMA
            w1_sb = [
                consts.tile([128, 16, H], F16, tag=f"w1{h}", name=f"w1{h}")
                for h in range(2)
            ]
            w2_sb = consts.tile([128, 32, H], F16, tag="w2")
            # never written: garbage operand for dependency-free PE warm-up
            wscr_sb = consts.tile([128, 2, 256], F16, tag="wscr")
            bia_sb = consts.tile([128, 3, 2], F32, tag="bia")
            fcw_sb = consts.tile([128, 4], F32, tag="fcw")
            fcb_sb = consts.tile([1, 1], F32, tag="fcb")

            # per-item d-sums of the relu'd s-half channels, [chan, chunk, item]
            s_sb = spool.tile([128, 4, B_CORE], F32, tag="s")

            # pipeline state (python-side references to live tiles)
            bg_t, xqa_t, xqb_t = {}, {}, {}
            z0_t, z1_t, z2_t = {}, {}, {}
            h1_t, h2_t = {}, {}
            ry_t = {}  # (g, chunk) -> tile; chunks 0..3 = L0oc0, L1oc0, L2oc0, L2oc1
            ps01_t, ps12_t = {}, {}  # psum pairs for L0, and for L1/L2

            def dma_xq(g, eng=None):
                for name, src, dst_map in (("xqa", xqa, xqa_t), ("xqb", xqb, xqb_t)):
                    t = xqpool.tile([128, C0, GROUP, D], F16, tag=name)
                    ap = bass.AP(
                        tensor=src,
                        offset=g * 128 * XQF,
                        ap=[[XQF, 128], [1, XQF]],
                    )
                    (eng or nc.sync).dma_start(t[:], ap)
                    dst_map[g] = t

            def dma_bg_half(g, mh):
                # half-m DMAs: regional dep tracking lets the first z chunks
                # start after only half the broadcast has landed
                if mh == 0:
                    bg_t[g] = bgpool.tile(
                        [128, GROUP, M, D], F16, tag="B", name="B"
                    )
                src = bass.AP(
                    tensor=xh,
                    offset=g * GROUP * MD + mh * 16 * D,
                    ap=[[0, 128], [MD, GROUP], [1, 16 * D]],
                )
                nc.sync.dma_start(
                    bg_t[g][:, :, 16 * mh : 16 * (mh + 1), :], src
                )

            def dma_bg(g):
                dma_bg_half(g, 0)
                dma_bg_half(g, 1)

            def build_z0(g):
                t = z0pool.tile([128, C0, GROUP, D], F16, tag="z0")
                nc.vector.tensor_mul(t[:], xqa_t[g][:], xqb_t[g][:])
                z0_t[g] = t
                del xqa_t[g], xqb_t[g]

            def mm_l0(g):
                ps = [
                    ppool.tile([128, GROUP * D], F32, tag="ps", name="ps0")
                    for _ in range(2)
                ]
                for c in range(C0):
                    for oc in range(2):
                        nc.tensor.matmul(
                            ps[oc][:],
                            w0s_sb[:, c, oc * 128 : (oc + 1) * 128],
                            z0_t[g][:, c, :, :],
                            start=(c == 0),
                            stop=(c == C0 - 1),
                        )
                ps01_t[g] = ps
                del z0_t[g]

            def act_l0(g):
                ps = ps01_t[g]
                h = hpool.tile([128, GROUP, D], F16, tag="h1")
                nc.scalar.activation(h[:], ps[1][:], RELU, bias=bia_sb[:, 0, 1:2])
                h1_t[g] = h
                r = rypool.tile([128, GROUP, D], F16, tag="ry")
                nc.scalar.activation(r[:], ps[0][:], RELU, bias=bia_sb[:, 0, 0:1])
                ry_t[(g, 0)] = r
                del ps01_t[g]

            # m-chunk widths per TT op; layer 2's first chunk is small so the
            # first L2 matmul's z arrives ~0.7us sooner after the h2 ACT
            # (the h2 -> z2-chunk0 chain must fit inside the 10-MM L0 burst).
            Z1_PLAN = (4, 4, 4, 4, 4, 4, 4, 4)
            Z2_PLAN = (2, 4, 4, 4, 4, 4, 4, 4, 2)

            def build_z12(g, lay):
                h = h1_t[g] if lay == 1 else h2_t[g]
                plan = Z1_PLAN if lay == 1 else Z2_PLAN
                tiles = []
                m0 = 0
                for w in plan:
                    pool = zpool if w == 4 else zspool
                    zt = pool.tile([128, GROUP, w, D], F16, tag=f"z{w}")
                    nc.vector.tensor_mul(
                        zt[:],
                        h[:, :, None, :].to_broadcast((128, GROUP, w, D)),
                        bg_t[g][:, :, m0 : m0 + w, :],
                    )
                    tiles.append(zt)
                    m0 += w
                if lay == 1:
                    z1_t[g] = tiles
                    del h1_t[g]
                else:
                    z2_t[g] = tiles
                    del h2_t[g]

            def mm_l12(g, lay):
                plan = Z1_PLAN if lay == 1 else Z2_PLAN
                tiles = z1_t[g] if lay == 1 else z2_t[g]
                ps = [
                    ppool.tile([128, GROUP * D], F32, tag="ps", name="ps12")
                    for _ in range(2)
                ]
                m = 0
                for zt, w in zip(tiles, plan):
                    for mm in range(w):
                        if lay == 1:
                            wt, mi = w1_sb[m // 16], m % 16
                        else:
                            wt, mi = w2_sb, m
                        for oc in range(2):
                            nc.tensor.matmul(
                                ps[oc][:],
                                wt[:, mi, oc * 128 : (oc + 1) * 128],
                                zt[:, :, mm, :],
                                start=(m == 0),
                                stop=(m == 31),
                            )
                        m += 1
                ps12_t[g] = ps
                if lay == 1:
                    del z1_t[g]
                else:
                    del z2_t[g]

            def act_l1(g):
                ps = ps12_t[g]
                h = hpool.tile([128, GROUP, D], F16, tag="h2")
                nc.scalar.activation(h[:], ps[1][:], RELU, bias=bia_sb[:, 1, 1:2])
                h2_t[g] = h
                r = rypool.tile([128, GROUP, D], F16, tag="ry")
                nc.scalar.activation(r[:], ps[0][:], RELU, bias=bia_sb[:, 1, 0:1])
                ry_t[(g, 1)] = r
                del ps12_t[g]

            def act_l2(g, fuse_red=False):
                ps = ps12_t[g]
                for oc in range(2):
                    r = rypool.tile([128, GROUP, D], F16, tag="ry")
                    nc.scalar.activation(
                        r[:], ps[oc][:], RELU, bias=bia_sb[:, 2, oc : oc + 1]
                    )
                    ry_t[(g, 2 + oc)] = r
                    if fuse_red:
                        # final group: reduce right behind each ACT so the
                        # closing FC chain pipelines across ScalarE/VectorE
                        red(g, 2 + oc)
                del ps12_t[g]

            def red(g, chunk):
                i0 = g * GROUP
                nc.vector.reduce_sum(
                    s_sb[:, chunk, i0 : i0 + GROUP],
                    ry_t[(g, chunk)][:],
                    axis=AX_X,
                )
                del ry_t[(g, chunk)]

            # ---------------- prologue ----------------
            # Concurrent DMA queues tank per-stream bandwidth on this part,
            # so issue ALL startup DMAs serially on sync in strict
            # consumption-priority order of the pipeline-fill chain.
            dma_xq(0)
            nc.sync.dma_start(w0s_sb[:], w0s[:, :, :])
            nc.sync.dma_start(bia_sb[:], bia[:])
            dma_bg_half(0, 0)
            nc.sync.dma_start(w1_sb[0][:], w1[:, 0:16, :])
            dma_bg_half(0, 1)
            nc.sync.dma_start(w1_sb[1][:], w1[:, 16:32, :])
            nc.sync.dma_start(w2_sb[:, 0:16, :], w2[:, 0:16, :])
            dma_xq(1)
            nc.sync.dma_start(w2_sb[:, 16:32, :], w2[:, 16:32, :])
            dma_bg(1)
            nc.sync.dma_start(fcw_sb[:], fcw[:])
            nc.sync.dma_start(fcb_sb[:], fcb[:])
            # PE warm-up: ~6us of garbage matmuls (into a scratch psum pair
            # that is never read) gated only on the first xq DMA, so the HAM
            # clock gate is at K=8/8 when the real layer-0 matmuls arrive.
            def pe_filler(n, stat, mov):
                wps = [
                    ppool.tile([128, GROUP * D], F32, tag="ps", name="warm")
                    for _ in range(2)
                ]
                for wi in range(n):
                    nc.tensor.matmul(
                        wps[wi % 2][:],
                        stat,
                        mov,
                        start=(wi < 2),
                        stop=(wi >= n - 2),
                    )

            nc.gpsimd.memset(wscr_sb[:], 0.0)
            pe_filler(16, wscr_sb[:, 0, 0:128], wscr_sb[:, :, :])
            build_z0(0)
            mm_l0(0)
            # keep the PE HAM-warm across the L0 -> first-L1 wait (gated on
            # the Bg0/w1 DMAs); this burst runs in otherwise-idle PE time
            pe_filler(24, wscr_sb[:, 0, 0:128], wscr_sb[:, :, :])
            act_l0(0)
            build_z12(0, 1)
            build_z0(1)

            # ---------------- steady-state pipeline ----------------
            # PE order per iter: [L1(i)][L0(i+1)][L2(i)]; DVE order:
            # [z2(i)][z1(i+1)][z0(i+2)][reduces] (reduces last — they are
            # not on the z critical chain that feeds the PE).
            for i in range(N_GROUPS):
                last = i == N_GROUPS - 1
                if i + 2 < N_GROUPS:
                    dma_xq(i + 2)
                mm_l12(i, 1)
                if last:
                    # drain the already-ready reduces before the z2 build so
                    # nothing queues ahead of the closing FC chain
                    red(i - 1, 2)
                    red(i - 1, 3)
                act_l1(i)
                build_z12(i, 2)
                if last:
                    red(i, 1)
                    pe_filler(10, w0s_sb[:, 0, 0:128], w0s_sb[:, 0:2, :])
                if not last:
                    mm_l0(i + 1)
                    act_l0(i + 1)
                    build_z12(i + 1, 1)
                if i + 2 < N_GROUPS:
                    dma_bg(i + 2)
                mm_l12(i, 2)
                act_l2(i, fuse_red=last)
                if i + 2 < N_GROUPS:
                    build_z0(i + 2)
                if not last:
                    if i >= 1:
                        red(i - 1, 2)
                        red(i - 1, 3)
                    red(i, 1)
                    if i == 0:
                        red(0, 0)
                    red(i + 1, 0)

            # ---------------- final FC ----------------
            fc_ps = fcp.tile([1, B_CORE], F32, tag="fc")
            for c in range(4):
                nc.tensor.matmul(
                    fc_ps[:],
                    fcw_sb[:, c : c + 1],
                    s_sb[:, c, :],
                    start=(c == 0),
                    stop=(c == 3),
                )
            osb = consts.tile([1, B_CORE], F32, tag="osb")
            nc.scalar.activation(osb[:], fc_ps[:], IDENT, bias=fcb_sb[0:1, 0:1])
            nc.sync.dma_start(out[:], osb[:])

    _legalize_waits(nc)
    return nc


def _legalize_waits(nc, max_waits=1):
    """walrus codegen allows at most 2 semaphore waits per instruction; spill
    the excess onto NoOps injected just before the offender on the same
    engine (same-engine FIFO makes this ordering-equivalent)."""
    for bb in nc.main_func.blocks:
        insts = bb.instructions
        new_list = []
        changed = False
        for ins in insts:
            si = ins.sync_info
            if si is not None and si.on_wait and len(si.on_wait) > max_waits:
                waits = list(si.on_wait)
                extra, keep = waits[:-max_waits], waits[-max_waits:]
                k = 0
                while k < len(extra):
                    chunk = extra[k : k + max_waits]
                    nop = mybir.InstNoOp(name=f"{ins.name}-w{k}", ins=[], outs=[])
                    nop.engine = ins.engine
                    nop.sync_info = mybir.SyncInfo(on_wait=chunk, on_update=[])
                    new_list.append(nop)
                    k += max_waits
                ins.sync_info = mybir.SyncInfo(
                    on_wait=keep,
                    on_update=list(si.on_update) if si.on_update else [],
                )
                changed = True
            new_list.append(ins)
        if changed:
            if hasattr(bb, "set_instructions"):
                bb.set_instructions(new_list)
            else:
                insts.clear()
                insts.extend(new_list)
                if len(bb.instructions) != len(new_list):
                    bb.instructions = new_list


def _sym_maps():
    """amap/bmap: pair index k' -> (a, b) with a <= b, padded to C0*128."""
    a, b = np.triu_indices(M)
    pad = C0 * 128 - NP0
    amap = np.concatenate([a, np.zeros(pad, np.int64)])
    bmap = np.concatenate([b, np.zeros(pad, np.int64)])
    return amap, bmap


def prep_inputs(x, W0, b0, W1, b1, W2, b2, fc_W, fc_b):
    """Host-side reshape/cast into the per-core input maps."""
    xh = np.ascontiguousarray(x.astype(np.float16))
    amap, bmap = _sym_maps()
    # xq[g-within-core, p, c, i, d] = xh[8g+i, amap[c*128+p], d], flattened to
    # [N_GROUPS, 128, C0*GROUP*D] per core (partition-major: one DMA per
    # group is 128 contiguous 5KB runs)
    idx_a = amap.reshape(C0, 128).T  # (128, C0)
    idx_b = bmap.reshape(C0, 128).T
    # gather -> (B_TOTAL, 128, C0, D), then regroup to (ncore*ngrp, 8, 128,
    # C0, D) and put the item axis between c and d
    def _hw(idx):
        g = xh[:, idx, :]  # (B_TOTAL, 128, C0, D)
        g = g.reshape(B_TOTAL // GROUP, GROUP, 128, C0, D)
        g = g.transpose(0, 2, 3, 1, 4)  # (groups, 128, C0, GROUP, D)
        return np.ascontiguousarray(g.reshape(B_TOTAL // GROUP, 128, C0 * GROUP * D))

    xqa = _hw(idx_a)
    xqb = _hw(idx_b)
    # fold W0 over symmetric pairs: rows a<b get W0[a,b]+W0[b,a]
    W0r = np.asarray(W0, np.float32).reshape(M, M, H)
    Wsym = W0r[amap[:NP0], bmap[:NP0]] + np.where(
        (amap[:NP0] != bmap[:NP0])[:, None], W0r[bmap[:NP0], amap[:NP0]], 0.0
    )
    Wpad = np.zeros((C0 * 128, H), np.float32)
    Wpad[:NP0] = Wsym
    # weights pretransposed to [k-part, chunk, H]
    w0s = np.ascontiguousarray(
        Wpad.astype(np.float16).reshape(C0, 128, H).transpose(1, 0, 2)
    )
    w1 = np.ascontiguousarray(
        W1.astype(np.float16).reshape(32, 128, H).transpose(1, 0, 2)
    )
    w2 = np.ascontiguousarray(
        W2.astype(np.float16).reshape(32, 128, H).transpose(1, 0, 2)
    )
    bia = np.ascontiguousarray(
        np.stack([b0, b1, b2]).reshape(3, 2, 128).transpose(2, 0, 1).astype(np.float32)
    )
    fcw = np.ascontiguousarray(fc_W.reshape(4, 128).T.astype(np.float32))
    fcb = np.ascontiguousarray(fc_b.reshape(1, 1).astype(np.float32))
    shared = {"w0s": w0s, "w1": w1, "w2": w2, "bia": bia, "fcw": fcw, "fcb": fcb}
    return [
        {
            "xh": xh[i * B_CORE : (i + 1) * B_CORE],
            "xqa": xqa[i * N_GROUPS : (i + 1) * N_GROUPS],
            "xqb": xqb[i * N_GROUPS : (i + 1) * N_GROUPS],
            **shared,
        }
        for i in range(N_CORES)
    ]


_NC = None


def _get_nc():
    global _NC
    if _NC is None:
        _NC = build()
    return _NC


def kernel(**inputs):
    in_maps = prep_inputs(**inputs)
    res = run_bass_kernel_spmd(_get_nc(), in_maps, list(range(N_CORES)))
    return np.ascontiguousarray(
        np.concatenate([r["out"] for r in res.results], axis=0).astype(np.float32)
    )
